# revision 23
# baseline (speedup 1.0000x reference)
"""GAT-style message passing kernel for Trainium2 (8 NeuronCores, data-parallel over batch).

Reference math (per sample, 2 layers, shared weights):
    hidden = x @ W_in + b_in                      # [N, H]
    per layer:
        xt  = hidden @ W_t + b_t
        s_j = xt @ a_j ; s_i = xt @ a_i           # xt only feeds the scores
        att = softmax_j(lrelu(s_i[i] + s_j[j]))
        hidden = att @ hidden + hidden

Restructurings used here:
 1) W_t folding: s = hidden @ (W_t a) + b_t.a  — the NxHxH transform collapses.
 2) Rank-21 factorization: hidden == U @ V with V = [W_in; b_in] constant and
    U0 = [x | 1];  per layer U <- att @ U + U  (attention commutes with V).
    All attention matmuls run on U's 21 columns; V is applied once at the end.
    The ones-column of U doubles per layer (att rows sum to 1), and its output
    row in E.T @ U equals 2^L * D — the softmax denominator comes for free.
 3) exp(lrelu(z)-C_i) = max(e^{z-C_i}, e^{0.01z-C_i}) and with C_i = s_i+maxS
    both branches are rank-1:  E[j,i] = max(p[j], p'[j]*g[i])  with
    p = e^{s_j-maxS}, p' = e^{0.01(s_j-maxS)}, g = e^{min(-0.99(s_i+maxS+c),80)}
    so the whole N^2 pass is ONE fused DVE tensor_scalar per tile, no N^2 exp.
 4) s for the next layer from the same product: s' = rD * (Y_U @ w21) + s.
"""

import numpy as np
from contextlib import ExitStack

S = 2          # samples per core
N = 2048
Din = 20
UD = Din + 1   # U columns: 20 x-features + ones
H = 128
NCH = 16       # j-chunks of 128
NB = 4         # i-blocks
FB = 512       # i-block width
NUM_LAYERS = 2
N_CORES = 8


def _build(ctx, tc, aps, ctot):
    import concourse.bass as bass
    from concourse import mybir
    from concourse.masks import make_identity

    nc = tc.nc
    f32 = mybir.dt.float32
    f16 = mybir.dt.float16
    Alu = mybir.AluOpType
    Act = mybir.ActivationFunctionType

    x_ap, w21_ap, v_ap, out_ap = aps

    consts = ctx.enter_context(tc.tile_pool(name="consts", bufs=1))
    utp = ctx.enter_context(tc.tile_pool(name="utp", bufs=2))        # U0T / YUT rows [UD, N]
    natp = ctx.enter_context(tc.tile_pool(name="natp", bufs=4))      # U_nat f32 [128, 16, UD]
    natp16 = ctx.enter_context(tc.tile_pool(name="natp16", bufs=4))  # U_nat fp16
    ynat = ctx.enter_context(tc.tile_pool(name="ynat", bufs=2))      # Ynat f32 [128, 16, UD]
    xin = ctx.enter_context(tc.tile_pool(name="xin", bufs=4))        # x load tiles
    gpool = ctx.enter_context(tc.tile_pool(name="gpool", bufs=3))    # gbc [128, 512]
    epool = ctx.enter_context(tc.tile_pool(name="epool", bufs=10))    # E tiles [128, 512] f16
    outp = ctx.enter_context(tc.tile_pool(name="outp", bufs=2))      # final hidden [128,16,128]
    small = ctx.enter_context(tc.tile_pool(name="small", bufs=12))
    psA = ctx.enter_context(tc.tile_pool(name="psA", bufs=2, space="PSUM"))  # ubc [128,512]
    psU = ctx.enter_context(tc.tile_pool(name="psU", bufs=3, space="PSUM"))  # YUT [UD,512]
    psT = ctx.enter_context(tc.tile_pool(name="psT", bufs=3, space="PSUM"))  # transposes

    ident = consts.tile([128, 128], f32)
    make_identity(nc, ident)
    ones_r = consts.tile([1, 128], f32)
    nc.vector.memset(ones_r, 1.0)
    w21_sb = consts.tile([UD, 2], f32)
    nc.sync.dma_start(out=w21_sb, in_=w21_ap)
    v_sb = consts.tile([UD, H], f32)
    nc.sync.dma_start(out=v_sb, in_=v_ap)

    def ts(out, in0, s1, s2, op0, op1=None):
        if op1 is None:
            nc.vector.tensor_scalar(out, in0, s1, None, op0)
        else:
            nc.vector.tensor_scalar(out, in0, s1, s2, op0, op1)

    # ------------- input stage: x -> U0 (natural + T), initial scores -------
    u_nat = [None, None]
    u_nat16 = [None, None]
    s_part = [None, None]   # biasless scores [128, 16, 2]
    for s in range(S):
        # one contiguous DMA; node n lives at (partition p, chunk c) with
        # n = 16 p + c — a fixed relabeling the attention sum is invariant to.
        xflat = xin.tile([128, NCH, Din], f32)
        nc.sync.dma_start(out=xflat, in_=x_ap[s].rearrange("(p c) d -> p c d", c=NCH))
        un = natp.tile([128, NCH, UD], f32, tag="unat")
        nc.vector.memset(un[:, :, Din:UD], 1.0)
        nc.vector.tensor_copy(un[:, :, 0:Din], xflat)
        u0t = utp.tile([UD, N], f32, tag="u0t")
        nc.vector.memset(u0t, 1.0)
        for c in range(NCH):
            pst = psT.tile([Din, 128], f32, tag="tp")
            nc.tensor.transpose(pst, xflat[:, c, :], ident)
            nc.scalar.copy(u0t[0:Din, c * 128:(c + 1) * 128], pst)
        un16 = natp16.tile([128, NCH, UD], f16, tag="unat16")
        nc.scalar.copy(un16, un)
        # initial biasless scores s0[j, c, z] = U0[j] . w21[:, z]
        pss = psT.tile([128, 32], f32, tag="tp")
        for c in range(NCH):
            nc.tensor.matmul(pss[:, 2 * c:2 * c + 2], lhsT=u0t[:, c * 128:(c + 1) * 128],
                             rhs=w21_sb, start=True, stop=True)
        s0 = small.tile([128, NCH, 2], f32, tag="s0")
        nc.vector.tensor_copy(s0, pss.rearrange("p (c z) -> p c z", z=2))
        u_nat[s], u_nat16[s], s_part[s] = un, un16, s0

    # ------------- layers ---------------------------------------------------
    for L in range(NUM_LAYERS):
        last = L == NUM_LAYERS - 1
        prep = {}
        # phase 1: per-sample score prep (small ops; PE only briefly)
        for s in range(S):
            un, un16, s0 = u_nat[s], u_nat16[s], s_part[s]
            m1 = small.tile([128, 1], f32, tag="m1")
            nc.vector.tensor_reduce(m1, s0[:, :, 0], axis=mybir.AxisListType.X, op=Alu.max)
            psm = psT.tile([1, 128], f32, tag="tp")
            nc.tensor.matmul(psm, lhsT=m1, rhs=ident, start=True, stop=True)
            m1r = small.tile([1, 128], f32, tag="m1r")
            nc.scalar.copy(m1r, psm)
            mx = small.tile([1, 1], f32, tag="mx")
            nc.vector.tensor_reduce(mx, m1r, axis=mybir.AxisListType.X, op=Alu.max)
            psmb = psT.tile([128, 1], f32, tag="tp")
            nc.tensor.matmul(psmb, lhsT=ones_r, rhs=mx, start=True, stop=True)
            maxbc = small.tile([128, 1], f32, tag="maxbc")
            nc.scalar.copy(maxbc, psmb)
            negmax = small.tile([128, 1], f32, tag="negmax")
            ts(negmax, maxbc, -1.0, None, Alu.mult)
            negmax001 = small.tile([128, 1], f32, tag="negmax001")
            ts(negmax001, maxbc, -0.01, None, Alu.mult)
            p_sb = small.tile([128, NCH], f32, tag="p_sb")
            nc.scalar.activation(p_sb, s0[:, :, 0], Act.Exp, bias=negmax[:, 0:1], scale=1.0)
            pp_sb = small.tile([128, NCH], f32, tag="pp_sb")
            nc.scalar.activation(pp_sb, s0[:, :, 0], Act.Exp, bias=negmax001[:, 0:1], scale=0.01)
            u1 = small.tile([128, NCH], f32, tag="u1")
            ts(u1, s0[:, :, 1], maxbc[:, 0:1], float(ctot), Alu.add, Alu.add)
            u_sb = small.tile([128, NCH], f32, tag="u_sb")
            ts(u_sb, u1, -0.99, 10.5, Alu.mult, Alu.min)
            g_row = small.tile([1, N], f16, tag="g_row")
            gbc = gpool.tile([128, N], f16, tag="gbc")
            for b in range(NB):
                psu = psA.tile([1, FB], f32, tag="urow")
                for k in range(4):
                    c = 4 * b + k
                    nc.tensor.transpose(psu[0:1, k * 128:(k + 1) * 128],
                                        u_sb[:, c:c + 1], ident)
                nc.scalar.activation(g_row[0:1, b * FB:(b + 1) * FB], psu, Act.Exp)
                nc.gpsimd.partition_broadcast(gbc[:, b * FB:(b + 1) * FB],
                                              g_row[0:1, b * FB:(b + 1) * FB])
            prep[s] = (p_sb, pp_sb, gbc)

        # phase 2: attention sweeps, both samples back-to-back on PE
        yuts = {}
        for s in range(S):
            p_sb, pp_sb, gbc = prep[s]
            un16 = u_nat16[s]
            yut_sb = utp.tile([UD, N], f32, tag="yut")
            NG = 3   # trailing chunks computed on GpSimd, emitted first
            for b in range(NB):
                yps = psU.tile([UD, FB], f32, tag="yps")
                etiles = {}
                for c in range(NCH - NG, NCH):
                    e_t = epool.tile([128, FB], f16, tag="e")
                    nc.gpsimd.tensor_scalar(e_t, gbc[:, b * FB:(b + 1) * FB],
                                            pp_sb[:, c:c + 1], p_sb[:, c:c + 1],
                                            Alu.mult, Alu.max)
                    etiles[c] = e_t
                for c in range(NCH):
                    if c not in etiles:
                        e_t = epool.tile([128, FB], f16, tag="e")
                        ts(e_t, gbc[:, b * FB:(b + 1) * FB], pp_sb[:, c:c + 1],
                           p_sb[:, c:c + 1], Alu.mult, Alu.max)
                        etiles[c] = e_t
                for c in range(NCH):
                    nc.tensor.matmul(yps, lhsT=un16[:, c, :], rhs=etiles[c],
                                     start=(c == 0), stop=(c == NCH - 1))
                nc.scalar.copy(yut_sb[:, b * FB:(b + 1) * FB], yps)
            yuts[s] = yut_sb

        # phase 3: normalize + residual (+ next scores or final output)
        for s in range(S):
            un, s0, yut_sb = u_nat[s], s_part[s], yuts[s]
            yn = ynat.tile([128, NCH, UD], f32, tag="ynat")
            for c in range(NCH):
                pst = psT.tile([128, UD], f32, tag="tp")
                nc.tensor.transpose(pst, yut_sb[:, c * 128:(c + 1) * 128],
                                    ident[0:UD, 0:UD])
                nc.scalar.copy(yn[:, c, :], pst)
            dsc = small.tile([128, NCH], f32, tag="dsc")
            ts(dsc, yn[:, :, Din], float(2.0 ** (-L)), None, Alu.mult)
            rd = small.tile([128, NCH], f32, tag="rd")
            nc.vector.reciprocal(rd, dsc)
            new_un = natp.tile([128, NCH, UD], f32, tag="unat")
            for c in range(NCH):
                nc.vector.scalar_tensor_tensor(new_un[:, c, :], yn[:, c, :],
                                               rd[:, c:c + 1], un[:, c, :],
                                               Alu.mult, Alu.add)
            if not last:
                new_un16 = natp16.tile([128, NCH, UD], f16, tag="unat16")
                nc.scalar.copy(new_un16, new_un)
                psq = psT.tile([128, 32], f32, tag="tp")
                for c in range(NCH):
                    nc.tensor.matmul(psq[:, 2 * c:2 * c + 2],
                                     lhsT=yut_sb[:, c * 128:(c + 1) * 128],
                                     rhs=w21_sb, start=True, stop=True)
                qp = small.tile([128, NCH, 2], f32, tag="qp")
                nc.scalar.copy(qp, psq.rearrange("p (c z) -> p c z", z=2))
                new_s0 = small.tile([128, NCH, 2], f32, tag="s0")
                for c in range(NCH):
                    nc.vector.scalar_tensor_tensor(new_s0[:, c, :], qp[:, c, :],
                                                   rd[:, c:c + 1], s0[:, c, :],
                                                   Alu.mult, Alu.add)
                u_nat[s], u_nat16[s], s_part[s] = new_un, new_un16, new_s0
            else:
                # hidden = U' @ V — pipelined: all transposes feed V-matmuls
                # through the now-idle sweep psum pools, copies alternate engines
                hout = outp.tile([128, NCH, H], f32, tag="hout")
                for c in range(NCH):
                    psut = psU.tile([UD, 128], f32, tag="yps")
                    nc.tensor.transpose(psut, new_un[:, c, :], ident)
                    u2t_c = small.tile([UD, 128], f32, tag="u2t")
                    nc.scalar.copy(u2t_c, psut)
                    psh = psT.tile([128, H], f32, tag="tp")
                    nc.tensor.matmul(psh, lhsT=u2t_c, rhs=v_sb, start=True, stop=True)
                    nc.vector.tensor_copy(hout[:, c, :], psh)
                nc.sync.dma_start(
                    out=out_ap[s].rearrange("(p c) h -> p c h", c=NCH),
                    in_=hout)

def _host_prep(inputs):
    x = np.ascontiguousarray(np.asarray(inputs["x"], dtype=np.float32))
    W_in = np.asarray(inputs["W_in"], dtype=np.float32)
    b_in = np.asarray(inputs["b_in"], dtype=np.float32)
    W_t = np.asarray(inputs["W_t"], dtype=np.float32)
    b_t = np.asarray(inputs["b_t"], dtype=np.float32)
    a = np.asarray(inputs["a"], dtype=np.float32)
    a_j, a_i = a[:H, 0], a[H:, 0]
    wj = (W_t @ a_j).astype(np.float32)
    wi = (W_t @ a_i).astype(np.float32)
    V = np.ascontiguousarray(np.concatenate([W_in, b_in[None, :]], axis=0))  # [21, 128]
    w21 = np.ascontiguousarray(np.stack([V @ wj, V @ wi], axis=1))           # [21, 2]
    ctot = float(np.float32(b_t @ a_j) + np.float32(b_t @ a_i))
    return x, w21, V, ctot


def build_program(ctot):
    import concourse.tile as tile
    from concourse import mybir
    from concourse.bacc import Bacc

    f32 = mybir.dt.float32
    nc = Bacc("TRN2", target_bir_lowering=False, debug=False)
    x_t = nc.dram_tensor("x", [S, N, Din], f32, kind="ExternalInput")
    w21_t = nc.dram_tensor("w21", [UD, 2], f32, kind="ExternalInput")
    v_t = nc.dram_tensor("v", [UD, H], f32, kind="ExternalInput")
    out_t = nc.dram_tensor("out", [S, N, H], f32, kind="ExternalOutput")
    aps = (x_t.ap(), w21_t.ap(), v_t.ap(), out_t.ap())
    with tile.TileContext(nc) as tc, ExitStack() as ctx:
        _build(ctx, tc, aps, ctot)
    nc.compile()
    return nc


def kernel(**inputs) -> np.ndarray:
    from concourse.bass_utils import run_bass_kernel_spmd

    x, w21, V, ctot = _host_prep(inputs)
    B = x.shape[0]
    nc = build_program(ctot)
    in_maps = []
    for i in range(N_CORES):
        in_maps.append({
            "x": np.ascontiguousarray(x[i * S:(i + 1) * S]),
            "w21": w21,
            "v": V,
        })
    res = run_bass_kernel_spmd(nc, in_maps, list(range(N_CORES)))
    out = np.concatenate([res.results[i]["out"] for i in range(N_CORES)], axis=0)
    assert out.shape == (B, N, H)
    return out
